# revision 26
# baseline (speedup 1.0000x reference)
"""Deformable conv2d (3x3, pad 1) on 8 trn2 NeuronCores.

Sharding: (batch b, image half) -> core 2*b + half. Each core:
  1. offset conv (PE matmuls over 2 c-tiles x 9 taps, N=400 position chunks)
  2. transpose offsets twice: position-major chunks (natural, for beta) and
     16-col slices (wrap layout, for gather indices); DVE index/bilinear math
  3. gather indices are computed DIRECTLY in the swdge 16-partition wrap
     layout and replicated to the 8 idx groups with 8 small sbuf DMAs
     (replaces the element-granular DRAM bounce of the old design)
  4. dma_gather of 2x2 fp16 pixel patches (all 256 ch) from an interleaved
     row-pair HBM image, positions-on-partitions
  5. bilinear combine folded into PE as diagonal-matmul accumulation
     (psum[c,p] += sum_j plane_j^T @ diag(beta_j)) -- diag matrices built
     4 per chunk with stride-0 broadcast views; transposes to channel-major
  6. main conv = 18-chunk PE accumulation over (c-tile, tap), + bias, store.
"""
import numpy as np

B, CIN, COUT, H, W = 4, 256, 256, 80, 80
NCORES = 8
HHALF = H // 2                      # 40 rows per core
NPOS = HHALF * W                    # 3200 positions per core
NCHUNK = NPOS // 128                # 25
NWORD = NPOS // 16                  # 200 wrap words
PITCH = 84                          # x2 pixel-group pitch per row
NGROUPS = PITCH * PITCH             # 7056 (rows in x2; 83*84 + slack)
FBIAS = 16.0                        # float->int truncation bias (floor trick)
CLIP_LO = 14.0                      # = -2 + FBIAS
CLIP_HI = 96.9                      # = 80.9 + FBIAS
FLAT_OFF = -(14 * PITCH + 14)       # flat = y0b*84 + x0b + FLAT_OFF
BLOCKS = [(0, 512), (512, 512), (1024, 512), (1536, 512),
          (2048, 512), (2560, 512), (3072, 128)]

_cached = {}


def _build_program():
    from concourse import bass, bacc, tile, mybir
    from contextlib import ExitStack

    fp16, fp32 = mybir.dt.float16, mybir.dt.float32
    i16, i32 = mybir.dt.int16, mybir.dt.int32
    A = mybir.AluOpType
    ACT_COPY = mybir.ActivationFunctionType.Copy
    ACT_IDENT = mybir.ActivationFunctionType.Identity

    nc = bacc.Bacc("TRN2", target_bir_lowering=False, debug=False,
                   num_devices=NCORES, num_swdge_queues=4)

    x2_d = nc.dram_tensor("x2", [NGROUPS, 512], fp16, kind="ExternalInput")
    xcf_d = nc.dram_tensor("xcf", [128, 2, 46 * PITCH], fp16, kind="ExternalInput")
    offw_d = nc.dram_tensor("offw", [128, 2, 9, 18], fp16, kind="ExternalInput")
    offb_d = nc.dram_tensor("offb", [18, 1], fp32, kind="ExternalInput")
    convw_d = nc.dram_tensor("convw", [128, 2, 9, 256], fp16, kind="ExternalInput")
    convbT_d = nc.dram_tensor("convbT", [128, 2], fp32, kind="ExternalInput")
    cyb_d = nc.dram_tensor("cyb", [128, NCHUNK, 9], fp32, kind="ExternalInput")
    cxb_d = nc.dram_tensor("cxb", [128, NCHUNK, 9], fp32, kind="ExternalInput")
    cybw_d = nc.dram_tensor("cybw", [16, 9, NWORD], fp32, kind="ExternalInput")
    cxbw_d = nc.dram_tensor("cxbw", [16, 9, NWORD], fp32, kind="ExternalInput")
    id16_d = nc.dram_tensor("id16", [128, 128], fp16, kind="ExternalInput")
    id32_d = nc.dram_tensor("id32", [18, 18], fp32, kind="ExternalInput")
    out_d = nc.dram_tensor("out", [2, 128, NPOS], fp16, kind="ExternalOutput")

    # overlapping gather-source view: [NGROUPS-1, 1024] with row stride 512
    x2_view = x2_d.ap().copy()
    v = x2_view.ap
    v[0] = [512, NGROUPS - 1]
    v[1] = [1, 1024]
    x2_view.ap = v

    def revec(ap, dims, extra_offset=0):
        """Rebuild an AP's dim list: dims = [(stride, num), ...]."""
        a = ap.copy()
        vv = a.ap
        while len(vv) > 1:
            vv.pop()
        vv[0] = list(dims[0])
        for d in dims[1:]:
            vv.append(list(d))
        a.ap = vv
        a.offset = a.offset + extra_offset
        return a

    with tile.TileContext(nc) as tc:
        with ExitStack() as ctx:
            persist = ctx.enter_context(tc.tile_pool(name="persist", bufs=1))
            idxws = [persist.tile([128, 9, npos // 16], i16, tag=f"idxw{j}",
                                  name=f"idxw{j}")
                     for j, (_, npos) in enumerate(BLOCKS)]
            flats = [persist.tile([16, 9, npos // 16], i16, tag=f"flat{j}",
                                  name=f"flat{j}")
                     for j, (_, npos) in enumerate(BLOCKS)]
            beta = persist.tile([128, NCHUNK, 9, 4], fp16)
            id16 = persist.tile([128, 128], fp16)
            convw = persist.tile([128, 2, 9, 256], fp16)
            convbT = persist.tile([128, 2], fp32)

            # dummy gather to pull in the gpsimd gather ucode NOW: the lib
            # swap barrier waits for all outstanding DMAs, so doing it later
            # (after the prologue floods the queues) stalls the first real
            # gather by tens of us.
            warmidx = persist.tile([128, 8], i16)
            warmout = persist.tile([128, 1, 1024], fp16)
            nc.vector.memset(warmidx[:], 0)
            nc.gpsimd.dma_gather(warmout[:], x2_view, warmidx[:],
                                 128, 128, 1024, elem_step=512, queue_num=0)

            # ---------------- prologue: offsets + indices ----------------
            with ExitStack() as pctx:
                ppool = pctx.enter_context(tc.tile_pool(name="pro", bufs=1))
                ppsum = pctx.enter_context(
                    tc.tile_pool(name="ppsum", bufs=2, space="PSUM"))

                # untouched pad at the pool head: the main loop's first gb
                # tiles land on these addresses with no WAR hazard, letting
                # block-0 gathers run before the prologue fully drains
                zpad = ppool.tile([128, 24576], fp16)
                # xcf in 4 overlapping 16-row bands so conv chunk rc only
                # waits for band rc//2 (rows [10b, 10b+16) of the 46-row map)
                xcfb = [ppool.tile([128, 2, 16 * PITCH], fp16, tag=f"xcf{b}",
                                   name=f"xcf{b}")
                        for b in range(4)]
                offw = ppool.tile([128, 2, 9, 18], fp16)
                offb = ppool.tile([18, 1], fp32)
                id32 = ppool.tile([18, 18], fp32)
                cyb = ppool.tile([128, NCHUNK, 9], fp32)
                cxb = ppool.tile([128, NCHUNK, 9], fp32)
                cybw = ppool.tile([16, 9, NWORD], fp32)
                cxbw = ppool.tile([16, 9, NWORD], fp32)
                off_sb = ppool.tile([18, NPOS], fp32)
                offT = ppool.tile([128, NCHUNK, 18], fp32)
                offTw = ppool.tile([16, NWORD, 18], fp32)
                nc.sync.dma_start(out=offw[:], in_=offw_d[:])
                nc.sync.dma_start(out=offb[:], in_=offb_d[:])
                nc.sync.dma_start(out=id32[:], in_=id32_d[:])
                nc.scalar.dma_start(out=cybw[:], in_=cybw_d[:])
                nc.scalar.dma_start(out=cxbw[:], in_=cxbw_d[:])
                nc.scalar.dma_start(out=cyb[:], in_=cyb_d[:])
                nc.scalar.dma_start(out=cxb[:], in_=cxb_d[:])
                # bands 0-1 on sync (finish early, before the idx dups need
                # the queue); bands 2-3 behind the small constants on scalar
                xr = lambda b: slice(10 * b * PITCH, (10 * b + 16) * PITCH)
                for b in range(4):
                    eng = nc.sync if b < 2 else nc.scalar
                    eng.dma_start(out=xcfb[b][:, 0], in_=xcf_d[:, 0, xr(b)])
                    eng.dma_start(out=xcfb[b][:, 1], in_=xcf_d[:, 1, xr(b)])
                # big weights after the latency-critical transfers
                nc.scalar.dma_start(out=convw[:], in_=convw_d[:])
                nc.scalar.dma_start(out=id16[:], in_=id16_d[:])
                nc.scalar.dma_start(out=convbT[:], in_=convbT_d[:])

                NE = NCHUNK * 9   # 225

                def sigma_pass(j0, j1, sfx):
                    """flat gather indices for blocks [j0, j1) in swdge wrap
                    layout. offTw[r, w, ch] = offsets for pos 16*w + r."""
                    w0 = BLOCKS[j0][0] // 16
                    w1 = (BLOCKS[j1 - 1][0] + BLOCKS[j1 - 1][1]) // 16
                    n = w1 - w0
                    dyw = revec(offTw[:], [(NWORD * 18, 16), (2, 9), (18, n)],
                                w0 * 18)
                    dxw = revec(offTw[:], [(NWORD * 18, 16), (2, 9), (18, n)],
                                w0 * 18 + 1)
                    cyv = revec(cybw[:], [(9 * NWORD, 16), (NWORD, 9), (1, n)], w0)
                    cxv = revec(cxbw[:], [(9 * NWORD, 16), (NWORD, 9), (1, n)], w0)
                    pyw = ppool.tile([16, 9, n], fp32, tag=f"pyw{sfx}")
                    pxw = ppool.tile([16, 9, n], fp32, tag=f"pxw{sfx}")
                    tw32 = ppool.tile([16, 9, n], i32, tag=f"tw32{sfx}")
                    y0w = ppool.tile([16, 9, n], fp32, tag=f"y0w{sfx}")
                    x0w = ppool.tile([16, 9, n], fp32, tag=f"x0w{sfx}")
                    gw = ppool.tile([16, 9, n], fp32, tag=f"gw{sfx}")
                    V = nc.vector
                    V.tensor_tensor(out=pyw[:], in0=dyw, in1=cyv, op=A.add)
                    V.tensor_tensor(out=pxw[:], in0=dxw, in1=cxv, op=A.add)
                    V.tensor_scalar(pyw[:], pyw[:], CLIP_LO, CLIP_HI, A.max, A.min)
                    V.tensor_scalar(pxw[:], pxw[:], CLIP_LO, CLIP_HI, A.max, A.min)
                    # robust floor (works under trunc or round-to-nearest cvt)
                    V.tensor_copy(tw32[:], pyw[:])
                    V.tensor_copy(y0w[:], tw32[:])
                    V.tensor_tensor(out=gw[:], in0=y0w[:], in1=pyw[:], op=A.is_gt)
                    V.tensor_tensor(out=y0w[:], in0=y0w[:], in1=gw[:], op=A.subtract)
                    V.tensor_copy(tw32[:], pxw[:])
                    V.tensor_copy(x0w[:], tw32[:])
                    V.tensor_tensor(out=gw[:], in0=x0w[:], in1=pxw[:], op=A.is_gt)
                    V.tensor_tensor(out=x0w[:], in0=x0w[:], in1=gw[:], op=A.subtract)
                    # flat = (y0b*84 + x0b) + FLAT_OFF
                    V.scalar_tensor_tensor(pyw[:], y0w[:], float(PITCH), x0w[:],
                                           A.mult, A.add)
                    V.tensor_scalar_add(pyw[:], pyw[:], float(FLAT_OFF))
                    for j in range(j0, j1):
                        b0 = BLOCKS[j][0] // 16 - w0
                        b1 = b0 + BLOCKS[j][1] // 16
                        V.tensor_copy(tw32[:, :, b0:b1], pyw[:, :, b0:b1])
                        V.tensor_copy(flats[j][:], tw32[:, :, b0:b1])

                def natural_pass(c0, c1):
                    """bilinear corner weights beta for chunk range [c0, c1)."""
                    nch = c1 - c0
                    e0, ne = c0 * 9, nch * 9
                    dy = revec(offT[:], [(NCHUNK * 18, 128), (18, nch), (2, 9)],
                               c0 * 18)
                    dx = revec(offT[:], [(NCHUNK * 18, 128), (18, nch), (2, 9)],
                               c0 * 18 + 1)
                    cyv = revec(cyb[:], [(NE, 128), (1, ne)], e0)
                    cxv = revec(cxb[:], [(NE, 128), (1, ne)], e0)
                    sl = lambda t: revec(t[:], [(NE, 128), (1, ne)], e0)
                    v3 = lambda t: revec(t[:], [(NE, 128), (9, nch), (1, 9)], e0)
                    V = nc.vector
                    V.tensor_tensor(out=sl(pyb), in0=dy, in1=cyv, op=A.add)
                    V.tensor_tensor(out=sl(pxb), in0=dx, in1=cxv, op=A.add)
                    V.tensor_scalar(sl(pyb), sl(pyb), CLIP_LO, CLIP_HI, A.max, A.min)
                    V.tensor_scalar(sl(pxb), sl(pxb), CLIP_LO, CLIP_HI, A.max, A.min)
                    V.tensor_copy(sl(t_i32), sl(pyb))
                    V.tensor_copy(sl(y0f), sl(t_i32))
                    V.tensor_tensor(out=sl(gtt), in0=sl(y0f), in1=sl(pyb), op=A.is_gt)
                    V.tensor_tensor(out=sl(y0f), in0=sl(y0f), in1=sl(gtt), op=A.subtract)
                    V.tensor_copy(sl(t_i32), sl(pxb))
                    V.tensor_copy(sl(x0f), sl(t_i32))
                    V.tensor_tensor(out=sl(gtt), in0=sl(x0f), in1=sl(pxb), op=A.is_gt)
                    V.tensor_tensor(out=sl(x0f), in0=sl(x0f), in1=sl(gtt), op=A.subtract)
                    V.tensor_tensor(out=sl(fy), in0=sl(pyb), in1=sl(y0f), op=A.subtract)
                    V.tensor_tensor(out=sl(fx), in0=sl(pxb), in1=sl(x0f), op=A.subtract)
                    V.tensor_scalar(sl(gy), sl(fy), -1.0, 1.0, A.mult, A.add)
                    V.tensor_scalar(sl(gx), sl(fx), -1.0, 1.0, A.mult, A.add)
                    # beta[j]: b0=gx*gy b1=gx*fy b2=fx*gy b3=fx*fy
                    bj = lambda j: revec(beta[:], [(NE * 4, 128), (36, nch), (4, 9)],
                                         c0 * 36 + j)
                    V.tensor_tensor(out=bj(0), in0=v3(gx), in1=v3(gy), op=A.mult)
                    V.tensor_tensor(out=bj(1), in0=v3(gx), in1=v3(fy), op=A.mult)
                    V.tensor_tensor(out=bj(2), in0=v3(fx), in1=v3(gy), op=A.mult)
                    V.tensor_tensor(out=bj(3), in0=v3(fx), in1=v3(fy), op=A.mult)

                pyb = ppool.tile([128, NE], fp32)
                pxb = ppool.tile([128, NE], fp32)
                t_i32 = ppool.tile([128, NE], i32)
                y0f = ppool.tile([128, NE], fp32)
                x0f = ppool.tile([128, NE], fp32)
                fy = ppool.tile([128, NE], fp32)
                fx = ppool.tile([128, NE], fp32)
                gy = ppool.tile([128, NE], fp32)
                gx = ppool.tile([128, NE], fp32)
                gtt = ppool.tile([128, NE], fp32)
                # offset conv: 8 chunks x 5 rows x 80 cols (N=400), interleaved
                # with transposes; the index path runs per 512-pos block as
                # soon as its words exist, so gathers start ~30us in.
                # xcf rows [h0-2 .. h0+42) are relocated to [0..44) by host.
                XB = 16 * PITCH
                # after conv chunk rc, emit sigma for these blocks / beta chunks
                sig_at = {1: (0, 1, "A"), 3: (1, 3, "B"), 6: (3, 5, "C"),
                          7: (5, 7, "D")}
                nat_at = {4: (0, 13), 6: (13, 21), 7: (21, NCHUNK)}
                nat_done = 0
                for rc in range(8):
                    ps = ppsum.tile([18, 400], fp32, tag="offps")
                    xcf = xcfb[rc // 2]
                    mm = 0
                    for ct in range(2):
                        for t in range(9):
                            ky, kx = t // 3, t % 3
                            base = ((rc % 2) * 5 + ky + 1) * PITCH + kx + 1
                            mov = revec(xcf[:, ct, 0],
                                        [(2 * XB, 128), (PITCH, 5), (1, 80)],
                                        extra_offset=base)
                            nc.tensor.matmul(
                                ps[:], lhsT=offw[:, ct, t, :], rhs=mov,
                                start=(mm == 0), stop=(mm == 17))
                            mm += 1
                    nc.scalar.activation(off_sb[:, rc * 400:(rc + 1) * 400], ps[:],
                                         ACT_IDENT, bias=offb[:])

                    # wrap transposes for this chunk: 16-col slices -> [16, 18]
                    ptw = ppsum.tile([16, 25 * 18], fp32, tag="offtw")
                    for t in range(25):
                        w = rc * 25 + t
                        nc.tensor.matmul(
                            ptw[:, t * 18:(t + 1) * 18],
                            lhsT=off_sb[:, 16 * w:16 * (w + 1)],
                            rhs=id32[:], start=True, stop=True,
                            is_transpose=True)
                    nc.scalar.activation(
                        revec(offTw[:], [(NWORD * 18, 16), (1, 25 * 18)],
                              rc * 25 * 18),
                        ptw[:], ACT_COPY)
                    # natural transposes now possible: [18, 128] -> [128, 18]
                    while (nat_done + 1) * 128 <= (rc + 1) * 400:
                        ch = nat_done
                        pt = ppsum.tile([128, 18], fp32, tag="offtps")
                        nc.tensor.matmul(pt[:],
                                         lhsT=off_sb[:, ch * 128:(ch + 1) * 128],
                                         rhs=id32[:], start=True, stop=True,
                                         is_transpose=True)
                        nc.scalar.activation(offT[:, ch, :], pt[:], ACT_COPY)
                        nat_done += 1

                    if rc in sig_at:
                        sigma_pass(*sig_at[rc])
                    if rc in nat_at:
                        natural_pass(*nat_at[rc])

            # ---------------- main loop ----------------
            with ExitStack() as mctx:
                mpool = mctx.enter_context(tc.tile_pool(name="main", bufs=1))
                mpsum = mctx.enter_context(
                    tc.tile_pool(name="mpsum", bufs=2, space="PSUM"))

                id16b = revec(id16[:], [(128, 128), (0, 36), (1, 128)])

                for bi, (base, npos) in enumerate(BLOCKS):
                    nsub = npos // 128
                    # replicate this block's flat indices to the 8 swdge idx
                    # groups; done here (not in the prologue) so the gathers'
                    # conservative queue waits only cover dups <= bi.
                    for g in range(8):
                        nc.sync.dma_start(
                            out=idxws[bi][16 * g:16 * (g + 1), :, :],
                            in_=flats[bi][:])
                    gbs = []
                    for k in range(9):
                        gb = mpool.tile([128, 4, 1024], fp16, tag=f"gb{k}",
                                        bufs=(2 if k < 8 else 1))
                        nc.gpsimd.dma_gather(
                            gb[:, :nsub, :], x2_view, idxws[bi][:, k, :],
                            npos, npos, 1024, elem_step=512,
                            queue_num=(bi * 9 + k) % 4)
                        gbs.append(gb)

                    valbuf = mpool.tile([128, 18, 512], fp16, tag="valbuf", bufs=1)
                    for s in range(nsub):
                        ch = base // 128 + s
                        # diag matrices for all 9 taps x 4 corners in ONE op:
                        # dt[p, k, j, m] = id16[p, m] * beta[p, ch, k, j]
                        dt = mpool.tile([128, 9, 4, 128], fp16, tag="dt", bufs=2)
                        betab = revec(beta[:], [(NE * 4, 128), (1, 36), (0, 128)],
                                      ch * 36)
                        nc.vector.tensor_tensor(
                            out=revec(dt[:], [(4608, 128), (128, 36), (1, 128)]),
                            in0=id16b, in1=betab, op=A.mult)
                        # bilinear combine + transpose to channel-major:
                        # 2 taps' (x 2 ct) results share one psum bank
                        for pi in range(5):
                            ks = [2 * pi] if pi == 4 else [2 * pi, 2 * pi + 1]
                            pv = mpsum.tile([128, 512], fp32, tag="pv", bufs=4)
                            for ki, k in enumerate(ks):
                                for ct in range(2):
                                    pw = pv[:, (ki * 2 + ct) * 128:
                                            (ki * 2 + ct + 1) * 128]
                                    for j in range(4):
                                        slot = (j // 2) * 4 + (j % 2) * 2 + ct
                                        nc.tensor.matmul(
                                            pw,
                                            lhsT=gbs[k][:, s, slot * 128:(slot + 1) * 128],
                                            rhs=dt[:, k, j, :],
                                            start=(j == 0), stop=(j == 3))
                            ncol = len(ks) * 2 * 128
                            dstv = revec(
                                valbuf[:],
                                [(18 * 512, 128), (512, len(ks) * 2), (1, 128)],
                                (4 * pi) * 512 + s * 128)
                            if pi == 4:
                                nc.vector.tensor_copy(dstv, pv[:, :ncol])
                            else:
                                nc.scalar.activation(dstv, pv[:, :ncol], ACT_COPY)

                    for ot in range(2):
                        po = mpsum.tile([128, 512], fp32, tag=f"po{ot}", bufs=2)
                        for ci in range(18):
                            k, ct = ci // 2, ci % 2
                            nc.tensor.matmul(
                                po[:, :npos],
                                lhsT=convw[:, ct, k, ot * 128:(ot + 1) * 128],
                                rhs=valbuf[:, ci, :npos],
                                start=(ci == 0), stop=(ci == 17))
                        osb = mpool.tile([128, 512], fp16, tag="osb", bufs=2)
                        nc.scalar.activation(osb[:, :npos], po[:, :npos],
                                             ACT_IDENT, bias=convbT[:, ot:ot + 1])
                        nc.scalar.dma_start(
                            out=out_d[ot, :, base:base + npos],
                            in_=osb[:, :npos])

    nc.compile()
    return nc


def _host_prep(x, offset_w, offset_b, conv_w, conv_b):
    """Build per-core input maps."""
    x = np.asarray(x, np.float32)
    offset_w = np.asarray(offset_w, np.float32)
    offset_b = np.asarray(offset_b, np.float32)
    conv_w = np.asarray(conv_w, np.float32)
    conv_b = np.asarray(conv_b, np.float32)

    # weights, shared
    # offset_w: [18, 256, 3, 3] -> [c128, ct, t, d]
    ow = offset_w.reshape(18, 2, 128, 3, 3)
    offw_h = np.ascontiguousarray(
        ow.reshape(18, 2, 128, 9).transpose(2, 1, 3, 0)).astype(np.float16)
    offb_h = offset_b.reshape(18, 1).astype(np.float32)
    cw = conv_w.reshape(256, 2, 128, 9)
    convw_h = np.ascontiguousarray(cw.transpose(2, 1, 3, 0)).astype(np.float16)  # [c,ct,t,o]
    convbT_h = np.ascontiguousarray(conv_b.reshape(2, 128).T).astype(np.float32)
    id16_h = np.eye(128, dtype=np.float16)
    id32_h = np.eye(18, dtype=np.float32)

    # per-core base constants
    k = np.arange(9)
    ry = (k // 3 - 1).astype(np.float32)
    rx = (k % 3 - 1).astype(np.float32)
    in_maps = []
    per_sample = {}
    for b in range(B):
        xc = np.ascontiguousarray(x[b].transpose(1, 2, 0))       # [H, W, C]
        xp = np.pad(xc, ((2, 2), (2, 2), (0, 0))).astype(np.float16)  # [84, 84, 256]
        x2 = np.zeros((PITCH, PITCH, 2, 256), np.float16)
        x2[:83, :, 0] = xp[:83]
        x2[:83, :, 1] = xp[1:84]
        x2_h = x2.reshape(NGROUPS, 512)
        per_sample[b] = (x2_h, xp)

    for core in range(NCORES):
        b, half = core // 2, core % 2
        h0 = half * HHALF
        x2_h, xp = per_sample[b]
        # xcf: channel-first, rows [h0-2 .. h0+42) of the padded image
        # relocated to local rows [0..44)
        xcf_rows = xp[h0:h0 + 44]                                # [44, 84, 256]
        xcf_full = np.zeros((46, PITCH, 256), np.float16)
        xcf_full[:44] = xcf_rows
        xcf_h = np.ascontiguousarray(
            xcf_full.transpose(2, 0, 1).reshape(2, 128, 46 * PITCH)
            .transpose(1, 0, 2))

        i = np.arange(NPOS)
        hloc = i // W
        wloc = i % W
        cyb_h = ((h0 + hloc)[:, None] + ry[None, :] + FBIAS).astype(np.float32)
        cxb_h = (wloc[:, None] + rx[None, :] + FBIAS).astype(np.float32)
        cyb_h = np.ascontiguousarray(
            cyb_h.reshape(NCHUNK, 128, 9).transpose(1, 0, 2))
        cxb_h = np.ascontiguousarray(
            cxb_h.reshape(NCHUNK, 128, 9).transpose(1, 0, 2))

        # wrap-layout base coords: position 16*w + r at [r, k, w]
        iw = (16 * np.arange(NWORD)[None, :] + np.arange(16)[:, None])  # [16, NWORD]
        hw_ = iw // W
        ww_ = iw % W
        cybw_h = np.ascontiguousarray(
            ((h0 + hw_)[:, None, :] + ry[None, :, None] + FBIAS)).astype(np.float32)
        cxbw_h = np.ascontiguousarray(
            (ww_[:, None, :] + rx[None, :, None] + FBIAS)).astype(np.float32)

        in_maps.append({
            "x2": x2_h, "xcf": xcf_h, "offw": offw_h, "offb": offb_h,
            "convw": convw_h, "convbT": convbT_h, "cyb": cyb_h, "cxb": cxb_h,
            "cybw": cybw_h, "cxbw": cxbw_h,
            "id16": id16_h, "id32": id32_h,
        })
    return in_maps


def kernel(x, offset_w, offset_b, conv_w, conv_b, _trace=False):
    from concourse.bass_utils import run_bass_kernel_spmd

    if "nc" not in _cached:
        _cached["nc"] = _build_program()
    nc = _cached["nc"]
    in_maps = _host_prep(x, offset_w, offset_b, conv_w, conv_b)
    res = run_bass_kernel_spmd(nc, in_maps, list(range(NCORES)), trace=_trace)
    _cached["last_result"] = res
    out = np.zeros((B, COUT, H, W), np.float32)
    for core in range(NCORES):
        b, half = core // 2, core % 2
        o = res.results[core]["out"]          # [2, 128, NPOS] fp16
        out[b, :, half * HHALF:(half + 1) * HHALF, :] = \
            o.astype(np.float32).reshape(COUT, HHALF, W)
    return out
